# revision 12
# baseline (speedup 1.0000x reference)
"""Trainium2 Bass kernel for the CompressiveEncoder problem.

Data-parallel over batch: each of the 8 NeuronCores processes one batch
element end-to-end (4 transformer layers with compressive-memory FIFO
update + aux losses). All heavy compute runs in the transposed layout
x^T [D, T] so every matmul contracts over the partition dimension, and
matmul inputs are kept in float32r (TF32) for full-rate PE throughput.

Self-contained: hardcodes all shapes; host does embedding gather,
weight repacking, TF32 pre-rounding, shard/unshard, and the final loss
scaling. No files outside this module are read.
"""

import math
import os
import sys
import types
from contextlib import ExitStack

import numpy as np

import concourse.bass as bass
import concourse.tile as tile
from concourse import bacc, mybir
from concourse.bass_utils import run_bass_kernel_spmd
from concourse.masks import make_identity

# Problem dims
N = 4
H = 8
D = 512
DH = 64
T = 512
MEM = 512
CMEM = 128
RATIO = 4
DFF = 2048
VOCAB = 32000
B = 8
EPS = 1e-5

P = 128
NC_D = D // P          # 4 d-chunks
NFC = DFF // P         # 16 ff chunks
KV = CMEM + MEM + T    # 1152
NJ = KV // P           # 9 kv chunks
NCORES = 8

F32 = mybir.dt.float32
F32R = mybir.dt.float32r
AF = mybir.ActivationFunctionType
ALU = mybir.AluOpType


def _round_tf32(x):
    x = np.ascontiguousarray(x, dtype=np.float32)
    u = x.view(np.uint32)
    r = ((u.astype(np.uint64) + 0x1000) & 0xFFFFE000).astype(np.uint32)
    return r.view(np.float32)


def _install_ntff_hook():
    import antenv
    if "antenv.axon_hooks" in sys.modules:
        return
    try:
        from trn_agent_boot import trn_boot
        hook = trn_boot._ntff_profile_via_ctypes("/opt/axon/libaxon_pjrt.so")
    except Exception:
        hook = None
    mod = types.ModuleType("antenv.axon_hooks")
    mod.get_axon_ntff_profile_hook = lambda: hook
    mod.set_axon_ntff_profile_hook = lambda h: None
    sys.modules["antenv.axon_hooks"] = mod
    antenv.axon_hooks = mod


def _positional_encoding():
    pos = np.arange(T, dtype=np.float64)[:, None]
    div = np.exp(np.arange(0, D, 2, dtype=np.float64) * (-math.log(10000.0) / D))
    ang = pos * div
    pe = np.stack([np.sin(ang), np.cos(ang)], axis=-1).reshape(T, D)
    return pe.astype(np.float32)


# ---------------------------------------------------------------------------
# Device program
# ---------------------------------------------------------------------------

def _build_program():
    nc = bacc.Bacc("TRN2", debug=False)

    def din(name, shape, dt=F32R):
        return nc.declare_dram_parameter(name, list(shape), dt, isOutput=False)

    def dout(name, shape, dt=F32R):
        return nc.declare_dram_parameter(name, list(shape), dt, isOutput=True)

    xT0 = din("xT0", [D, T])
    memT = din("memT", [N, D, MEM])
    cmemT = din("cmemT", [N, D, CMEM])
    wq = din("wq", [N, D, D])
    wk = din("wk", [N, D, D])
    wv = din("wv", [N, D, D])
    wo = din("wo", [N, D, D])
    cwm = din("cwm", [N, RATIO * D, D])
    dwm = din("dwm", [N, CMEM, MEM])
    w1m = din("w1m", [N, D, DFF])
    w2m = din("w2m", [N, DFF, D])
    cbm = din("cbm", [N, 1, D])
    dbm = din("dbm", [N, 1, MEM])
    gb1 = din("gb1", [N, 2, D])           # rows: [-?] g then b (f32r, lhsT of Db matmul)
    gb2 = din("gb2", [N, 2, D])
    g1c = din("g1c", [N, D], F32)         # per-partition scalar layout
    g2c = din("g2c", [N, D], F32)
    boc = din("boc", [N, D], F32)
    b1c = din("b1c", [N, DFF], F32)
    b2c = din("b2c", [N, D], F32)

    yT_out = dout("yT_out", [N, D, T])
    comp_out = dout("comp_out", [N, CMEM, D])
    xT_out = dout("xT_out", [D, T])
    loss_out = dout("loss_out", [1, 2], F32)

    with tile.TileContext(nc) as tc, ExitStack() as ctx:
        ctx.enter_context(nc.allow_low_precision(
            reason="float32r (tf32) matmul inputs are intentional"))
        const = ctx.enter_context(tc.tile_pool(name="const", bufs=1))
        wgt = ctx.enter_context(tc.tile_pool(name="wgt", bufs=1))
        wb4 = ctx.enter_context(tc.tile_pool(name="wb4", bufs=3))
        data = ctx.enter_context(tc.tile_pool(name="data", bufs=1))
        sps = ctx.enter_context(tc.tile_pool(name="sps", bufs=1, space="PSUM"))

        # --- constants ---
        ones_col = const.tile([P, 1], F32R)
        nc.vector.memset(ones_col.bitcast(F32), 1.0)
        ones_row = const.tile([1, P], F32R)
        nc.vector.memset(ones_row.bitcast(F32), 1.0)
        ident = const.tile([P, P], F32)
        make_identity(nc, ident)
        ones_col32 = const.tile([P, 1], F32)
        nc.vector.memset(ones_col32, 1.0)
        eps_t = const.tile([1, 1], F32)
        nc.vector.memset(eps_t, EPS)
        aux_sse = const.tile([1, N], F32)
        ae_sse = const.tile([1, N], F32)

        # --- initial x ---
        xs = []
        for c in range(NC_D):
            x_c = data.tile([P, T], F32R, name="x_c", tag="x", bufs=6)
            nc.sync.dma_start(out=x_c, in_=xT0.ap()[c * P:(c + 1) * P, :])
            xs.append(x_c)

        def layernorm(xs, gbt, gcol, lname):
            """LayerNorm over partition (D) dim of x^T -> list of 4 f32r chunks."""
            psA = sps.tile([1, T], F32, name="psA", tag="rot", bufs=3)
            psB = sps.tile([1, T], F32, name="psB", tag="rot", bufs=3)
            for c in range(NC_D):
                xsq = data.tile([P, T], F32R, name="xsq", tag="xsq", bufs=1)
                nc.vector.tensor_tensor(out=xsq, in0=xs[c], in1=xs[c], op=ALU.mult)
                nc.tensor.matmul(psA, ones_col, xs[c], start=(c == 0), stop=(c == 3))
                nc.tensor.matmul(psB, ones_col, xsq, start=(c == 0), stop=(c == 3))
            lnrows = data.tile([97, T], F32, name="lnrows", tag="lnrows", bufs=1)
            mu = lnrows[0:1, :]
            musq = lnrows[32:33, :]
            var = lnrows[64:65, :]
            lnv = lnrows[96:97, :]
            nc.scalar.activation(mu, psA, AF.Copy, scale=1.0 / D)
            nc.vector.tensor_tensor(out=musq, in0=mu, in1=mu, op=ALU.mult)
            nc.vector.scalar_tensor_tensor(
                out=var, in0=psB, scalar=1.0 / D, in1=musq, op0=ALU.mult, op1=ALU.subtract)
            nc.scalar.activation(lnv, var, AF.Ln, bias=eps_t)
            rstd = data.tile([1, T], F32R, name="rstd", tag="rstd", bufs=1)
            nc.scalar.activation(rstd, lnv, AF.Exp, scale=-0.5)
            bb = data.tile([2, T], F32R, name="bb", tag="bb", bufs=1)
            nc.vector.memset(bb.bitcast(F32), 1.0)
            nc.vector.scalar_tensor_tensor(
                out=bb[0:1, :], in0=mu, scalar=-1.0, in1=rstd, op0=ALU.mult, op1=ALU.mult)
            a_b = sps.tile([P, T], F32, name="a_b", tag="rot", bufs=3)
            nc.tensor.matmul(a_b, ones_row, rstd, start=True, stop=True)
            ys = []
            for c in range(NC_D):
                d_b = sps.tile([P, T], F32, name="d_b", tag="rot", bufs=3)
                nc.tensor.matmul(d_b, gbt[:, c, :], bb, start=True, stop=True)
                tmp = data.tile([P, T], F32, name="lntmp", tag="lntmp", bufs=1)
                nc.vector.tensor_tensor(out=tmp, in0=xs[c], in1=a_b, op=ALU.mult)
                y_c = data.tile([P, T], F32R, name=f"y_{lname}", tag="y", bufs=6)
                nc.vector.scalar_tensor_tensor(
                    out=y_c, in0=tmp, scalar=gcol[:, c:c + 1],
                    in1=d_b, op0=ALU.mult, op1=ALU.add)
                ys.append(y_c)
            return ys

        for l in range(N):
            # ---- per-layer weight loads ----
            wqt = wgt.tile([P, NC_D, D], F32R, name="wqt", tag="wqt", bufs=1)
            nc.sync.dma_start(out=wqt, in_=wq.ap()[l].rearrange("(c p) f -> p c f", p=P))
            wkt = wgt.tile([P, NC_D, D], F32R, name="wkt", tag="wkt", bufs=1)
            nc.sync.dma_start(out=wkt, in_=wk.ap()[l].rearrange("(c p) f -> p c f", p=P))
            wvt = wgt.tile([P, NC_D, D], F32R, name="wvt", tag="wvt", bufs=1)
            nc.sync.dma_start(out=wvt, in_=wv.ap()[l].rearrange("(c p) f -> p c f", p=P))
            wot = wgt.tile([P, NC_D, D], F32R, name="wot", tag="wot", bufs=1)
            nc.sync.dma_start(out=wot, in_=wo.ap()[l].rearrange("(c p) f -> p c f", p=P))
            dwt = wgt.tile([P, MEM], F32R, name="dwt", tag="dwt", bufs=1)
            nc.sync.dma_start(out=dwt, in_=dwm.ap()[l])
            cb_row = wgt.tile([1, D], F32R, name="cb_row", tag="cb_row", bufs=2)
            nc.sync.dma_start(out=cb_row, in_=cbm.ap()[l])
            db_row = wgt.tile([1, MEM], F32R, name="db_row", tag="db_row", bufs=2)
            nc.sync.dma_start(out=db_row, in_=dbm.ap()[l])
            gb1t = wgt.tile([2, NC_D, P], F32R, name="gb1t", tag="gb1t", bufs=2)
            nc.sync.dma_start(out=gb1t, in_=gb1.ap()[l].rearrange("g (c p) -> g c p", p=P))
            gb2t = wgt.tile([2, NC_D, P], F32R, name="gb2t", tag="gb2t", bufs=2)
            nc.sync.dma_start(out=gb2t, in_=gb2.ap()[l].rearrange("g (c p) -> g c p", p=P))
            g1t = wgt.tile([P, NC_D], F32, name="g1t", tag="g1t", bufs=2)
            nc.sync.dma_start(out=g1t, in_=g1c.ap()[l].rearrange("(c p) -> p c", p=P))
            g2t = wgt.tile([P, NC_D], F32, name="g2t", tag="g2t", bufs=2)
            nc.sync.dma_start(out=g2t, in_=g2c.ap()[l].rearrange("(c p) -> p c", p=P))
            bot = wgt.tile([P, NC_D], F32, name="bot", tag="bot", bufs=2)
            nc.sync.dma_start(out=bot, in_=boc.ap()[l].rearrange("(c p) -> p c", p=P))
            b1t = wgt.tile([P, NFC], F32, name="b1t", tag="b1t", bufs=2)
            nc.sync.dma_start(out=b1t, in_=b1c.ap()[l].rearrange("(c p) -> p c", p=P))
            b2t = wgt.tile([P, NC_D], F32, name="b2t", tag="b2t", bufs=2)
            nc.sync.dma_start(out=b2t, in_=b2c.ap()[l].rearrange("(c p) -> p c", p=P))

            memt = data.tile([P, NC_D, MEM], F32R, name="memt", tag="memt", bufs=1)
            nc.sync.dma_start(out=memt, in_=memT.ap()[l].rearrange("(c p) m -> p c m", p=P))
            cmemt = data.tile([P, NC_D, CMEM], F32R, name="cmemt", tag="cmemt", bufs=1)
            nc.sync.dma_start(out=cmemt, in_=cmemT.ap()[l].rearrange("(c p) m -> p c m", p=P))

            # ---- LN1 -> y (also the new mem) ----
            ys = layernorm(xs, gb1t, g1t, "ln1")

            for c in range(NC_D):
                nc.sync.dma_start(
                    out=yT_out.ap()[l][c * P:(c + 1) * P, :], in_=ys[c])

            # ---- compressive path: comp, comp^T, ck^T, cv, recon/ae ----
            comp_ps = sps.tile([P, D], F32, name="comp_ps", tag="acc", bufs=4)
            first = True
            for r in range(RATIO):
                cwq = wb4.tile([P, NC_D, D], F32R, name="cwq", tag="wb4")
                nc.sync.dma_start(
                    out=cwq,
                    in_=cwm.ap()[l][r * D:(r + 1) * D, :].rearrange(
                        "(c p) o -> p c o", p=P))
                for c in range(NC_D):
                    lhs = memt[:, c, :].rearrange("p (lq r) -> p r lq", r=RATIO)[:, r, :]
                    nc.tensor.matmul(comp_ps, lhs, cwq[:, c, :],
                                     start=first, stop=False)
                    first = False
            nc.tensor.matmul(comp_ps, ones_row, cb_row, start=False, stop=True)
            comp_t = data.tile([P, D], F32R, name="comp_t", tag="comp", bufs=1)
            nc.any.tensor_copy(out=comp_t, in_=comp_ps)
            nc.sync.dma_start(out=comp_out.ap()[l], in_=comp_t)

            compT = data.tile([P, NC_D, CMEM], F32R, name="compT", tag="compT", bufs=1)
            for c in range(NC_D):
                tps = sps.tile([P, P], F32, name="tps", tag="rot", bufs=3)
                nc.tensor.transpose(tps, comp_t[:, c * P:(c + 1) * P].bitcast(F32), ident)
                nc.any.tensor_copy(out=compT[:, c, :], in_=tps)

            ckT = data.tile([P, NC_D, CMEM], F32R, name="ckT", tag="ckT", bufs=1)
            for fc in range(NC_D):
                ck_ps = sps.tile([P, CMEM], F32, name="ck_ps", tag="rot", bufs=3)
                for c in range(NC_D):
                    nc.tensor.matmul(ck_ps, wkt[:, c, fc * P:(fc + 1) * P], compT[:, c, :],
                                     start=(c == 0), stop=(c == 3))
                nc.any.tensor_copy(out=ckT[:, fc, :], in_=ck_ps)

            cvplus = data.tile([P, H, DH + 1], F32R, name="cvplus", tag="cvplus", bufs=1)
            nc.vector.memset(cvplus.bitcast(F32), 1.0)
            cv_ps = sps.tile([P, D], F32, name="cv_ps", tag="acc", bufs=4)
            for c in range(NC_D):
                nc.tensor.matmul(cv_ps, compT[:, c, :], wvt[:, c, :],
                                 start=(c == 0), stop=(c == 3))
            nc.any.tensor_copy(
                out=cvplus[:, :, 0:DH],
                in_=cv_ps.rearrange("p (h d) -> p h d", h=H))

            # recon + ae loss pieces
            acc_ae = data.tile([P, NC_D], F32, name="acc_ae", tag="acc_ae", bufs=2)
            for c in range(NC_D):
                rec_ps = sps.tile([P, MEM], F32, name="rec_ps", tag="rot", bufs=3)
                nc.tensor.matmul(rec_ps, comp_t[:, c * P:(c + 1) * P], dwt,
                                 start=True, stop=False)
                nc.tensor.matmul(rec_ps, ones_row, db_row, start=False, stop=True)
                dt_ = data.tile([P, MEM], F32, name="dt_", tag="dt", bufs=1)
                nc.vector.tensor_tensor(out=dt_, in0=memt[:, c, :], in1=rec_ps,
                                        op=ALU.subtract)
                scr = data.tile([P, MEM], F32, name="scr", tag="junk", bufs=2)
                nc.vector.scalar_tensor_tensor(
                    out=scr, in0=dt_, scalar=1.0, in1=dt_, op0=ALU.mult, op1=ALU.mult,
                    accum_out=acc_ae[:, c:c + 1])
            red_ae = data.tile([P, 1], F32, name="red_ae", tag="red_ae", bufs=2)
            nc.vector.tensor_reduce(out=red_ae, in_=acc_ae, axis=mybir.AxisListType.X,
                                    op=ALU.add)
            lp_ae = sps.tile([1, 1], F32, name="lp_ae", tag="rot", bufs=3)
            nc.tensor.matmul(lp_ae, red_ae, ones_col32, start=True, stop=True)
            nc.scalar.copy(ae_sse[0:1, l:l + 1], lp_ae)

            # ---- attention ----
            # Vplus tiles (v for all 1152 kv positions + ones column)
            vplus = []
            for jc in range(NJ):
                vp = data.tile([P, H, DH + 1], F32R, name="vp", tag="vplus", bufs=NJ)
                nc.vector.memset(vp.bitcast(F32), 1.0)
                v_ps = sps.tile([P, D], F32, name="v_ps", tag="acc", bufs=4)
                for c in range(NC_D):
                    if jc == 0:
                        lhs = cmemt[:, c, :]
                    elif jc <= 4:
                        lhs = memt[:, c, (jc - 1) * P:jc * P]
                    else:
                        lhs = ys[c][:, (jc - 5) * P:(jc - 4) * P]
                    nc.tensor.matmul(v_ps, lhs, wvt[:, c, :], start=(c == 0), stop=(c == 3))
                nc.any.tensor_copy(out=vp[:, :, 0:DH],
                                   in_=v_ps.rearrange("p (h d) -> p h d", h=H))
                vplus.append(vp)

            oall = []
            for c in range(NC_D):
                o_c = data.tile([P, T], F32R, name="o_c", tag="oall", bufs=4)
                oall.append(o_c)
            acc_aux = data.tile([DH, H], F32, name="acc_aux", tag="acc_aux", bufs=2)

            for fc in range(NC_D):
                # q^T and k^T for this head pair
                q_ps = sps.tile([P, T], F32, name="q_ps", tag="acc", bufs=4)
                for c in range(NC_D):
                    nc.tensor.matmul(q_ps, wqt[:, c, fc * P:(fc + 1) * P], ys[c],
                                     start=(c == 0), stop=(c == 3))
                qT = data.tile([P, T], F32R, name="qT", tag="qT", bufs=2)
                nc.any.tensor_copy(out=qT, in_=q_ps)

                kT = data.tile([P, KV], F32R, name="kT", tag="kT", bufs=1)
                kc_ps = sps.tile([P, CMEM], F32, name="kc_ps", tag="rot", bufs=3)
                km_ps = sps.tile([P, MEM], F32, name="km_ps", tag="acc", bufs=4)
                ky_ps = sps.tile([P, T], F32, name="ky_ps", tag="acc", bufs=4)
                for c in range(NC_D):
                    wslice = wkt[:, c, fc * P:(fc + 1) * P]
                    nc.tensor.matmul(kc_ps, wslice, cmemt[:, c, :], start=(c == 0), stop=(c == 3))
                    nc.tensor.matmul(km_ps, wslice, memt[:, c, :], start=(c == 0), stop=(c == 3))
                    nc.tensor.matmul(ky_ps, wslice, ys[c], start=(c == 0), stop=(c == 3))
                nc.any.tensor_copy(out=kT[:, 0:CMEM], in_=kc_ps)
                nc.any.tensor_copy(out=kT[:, CMEM:CMEM + MEM], in_=km_ps)
                nc.any.tensor_copy(out=kT[:, CMEM + MEM:KV], in_=ky_ps)

                for hh in range(2):
                    h = 2 * fc + hh
                    po = hh * DH
                    ps_m = sps.tile([DH + 1, T], F32, name="ps_m", tag="acc", bufs=4)
                    ps_cs = sps.tile([DH + 1, T], F32, name="ps_cs", tag="acc", bufs=4)
                    for jc in range(NJ):
                        dots = sps.tile([P, T], F32, name="dots", tag="rot", bufs=3)
                        nc.tensor.matmul(dots, kT[po:po + DH, jc * P:(jc + 1) * P],
                                         qT[po:po + DH, :], start=True, stop=True)
                        expt = data.tile([P, T], F32R, name="expt", tag="expt", bufs=3)
                        nc.scalar.activation(expt, dots, AF.Exp, scale=DH ** -0.5)
                        if 1 <= jc <= 4:
                            nc.tensor.matmul(ps_m, vplus[jc][:, h, :], expt,
                                             start=(jc == 1), stop=(jc == 4))
                        else:
                            nc.tensor.matmul(ps_cs, vplus[jc][:, h, :], expt,
                                             start=(jc == 0), stop=(jc == 8))
                    m_s = data.tile([DH + 1, T], F32, name="m_s", tag="as", bufs=3)
                    nc.any.tensor_copy(out=m_s, in_=ps_m)
                    sum3 = data.tile([DH + 1, T], F32, name="sum3", tag="as", bufs=3)
                    nc.vector.tensor_tensor(out=sum3, in0=m_s, in1=ps_cs, op=ALU.add)
                    r_tot = data.tile([1, T], F32R, name="r_tot", tag="r_tot", bufs=1)
                    nc.vector.reciprocal(out=r_tot, in_=sum3[DH:DH + 1, :])
                    r_mem = data.tile([1, T], F32R, name="r_mem", tag="r_mem", bufs=1)
                    nc.vector.reciprocal(out=r_mem, in_=m_s[DH:DH + 1, :])
                    rt_b = sps.tile([DH, T], F32, name="rt_b", tag="rot", bufs=3)
                    nc.tensor.matmul(rt_b, ones_row[:, 0:DH], r_tot, start=True, stop=True)
                    nc.vector.tensor_tensor(out=oall[fc][po:po + DH, :], in0=sum3[0:DH, :],
                                            in1=rt_b, op=ALU.mult)
                    rm_b = sps.tile([DH, T], F32, name="rm_b", tag="rot", bufs=3)
                    nc.tensor.matmul(rm_b, ones_row[:, 0:DH], r_mem, start=True, stop=True)
                    attn1 = data.tile([DH, T], F32, name="attn1", tag="ad", bufs=3)
                    nc.vector.tensor_tensor(out=attn1, in0=m_s[0:DH, :], in1=rm_b,
                                            op=ALU.mult)

                    # aux2: attention over compressed memory keys
                    dots2 = sps.tile([CMEM, T], F32, name="dots2", tag="rot", bufs=3)
                    nc.tensor.matmul(dots2, ckT[po:po + DH, fc, :], qT[po:po + DH, :],
                                     start=True, stop=True)
                    expt2 = data.tile([CMEM, T], F32R, name="expt2", tag="expt", bufs=3)
                    nc.scalar.activation(expt2, dots2, AF.Exp, scale=DH ** -0.5)
                    ps2 = sps.tile([DH + 1, T], F32, name="ps2", tag="acc", bufs=4)
                    nc.tensor.matmul(ps2, cvplus[:, h, :], expt2, start=True, stop=True)
                    a2 = data.tile([DH + 1, T], F32, name="a2", tag="as", bufs=3)
                    nc.any.tensor_copy(out=a2, in_=ps2)
                    r2 = data.tile([1, T], F32R, name="r2", tag="r2", bufs=1)
                    nc.vector.reciprocal(out=r2, in_=a2[DH:DH + 1, :])
                    r2_b = sps.tile([DH, T], F32, name="r2_b", tag="rot", bufs=3)
                    nc.tensor.matmul(r2_b, ones_row[:, 0:DH], r2, start=True, stop=True)
                    attn2 = data.tile([DH, T], F32, name="attn2", tag="ad", bufs=3)
                    nc.vector.tensor_tensor(out=attn2, in0=a2[0:DH, :], in1=r2_b,
                                            op=ALU.mult)
                    dh_t = data.tile([DH, T], F32, name="dh_t", tag="ad", bufs=3)
                    nc.vector.tensor_tensor(out=dh_t, in0=attn1, in1=attn2, op=ALU.subtract)
                    scr2 = data.tile([DH, T], F32, name="scr2", tag="junk", bufs=2)
                    nc.vector.scalar_tensor_tensor(
                        out=scr2, in0=dh_t, scalar=1.0, in1=dh_t, op0=ALU.mult,
                        op1=ALU.mult, accum_out=acc_aux[:, h:h + 1])

            red_aux = data.tile([DH, 1], F32, name="red_aux", tag="red_aux", bufs=2)
            nc.vector.tensor_reduce(out=red_aux, in_=acc_aux, axis=mybir.AxisListType.X,
                                    op=ALU.add)
            lp_aux = sps.tile([1, 1], F32, name="lp_aux", tag="rot", bufs=3)
            nc.tensor.matmul(lp_aux, red_aux, ones_col32[0:DH, :], start=True, stop=True)
            nc.scalar.copy(aux_sse[0:1, l:l + 1], lp_aux)

            # ---- wo projection + residual ----
            xs2 = []
            for ec in range(NC_D):
                z_ps = sps.tile([P, T], F32, name="z_ps", tag="acc", bufs=4)
                for c in range(NC_D):
                    nc.tensor.matmul(z_ps, wot[:, c, ec * P:(ec + 1) * P], oall[c],
                                     start=(c == 0), stop=(c == 3))
                x2_c = data.tile([P, T], F32R, name="x2_c", tag="x", bufs=6)
                nc.vector.scalar_tensor_tensor(
                    out=x2_c, in0=z_ps, scalar=bot[:, ec:ec + 1], in1=xs[ec],
                    op0=ALU.add, op1=ALU.add)
                xs2.append(x2_c)

            # ---- LN2 + FF ----
            y2s = layernorm(xs2, gb2t, g2t, "ln2")
            xs3 = []
            z2_ps = []
            for ec in range(NC_D):
                z2 = sps.tile([P, T], F32, name="z2", tag="acc", bufs=4)
                z2_ps.append(z2)
            for g in range(4):
                w1q = wb4.tile([P, NC_D, D], F32R, name="w1q", tag="wb4")
                nc.sync.dma_start(
                    out=w1q,
                    in_=w1m.ap()[l][:, g * D:(g + 1) * D].rearrange(
                        "(c p) f -> p c f", p=P))
                w2q = wb4.tile([P, NC_D, D], F32R, name="w2q", tag="wb4")
                nc.sync.dma_start(
                    out=w2q,
                    in_=w2m.ap()[l][g * D:(g + 1) * D, :].rearrange(
                        "(c p) f -> p c f", p=P))
                uts = []
                for k in range(4):
                    fc = g * 4 + k
                    u_ps = sps.tile([P, T], F32, name="u_ps", tag="rot", bufs=3)
                    for c in range(NC_D):
                        nc.tensor.matmul(u_ps, w1q[:, c, k * P:(k + 1) * P], y2s[c],
                                         start=(c == 0), stop=(c == 3))
                    ut = data.tile([P, T], F32R, name="ut", tag="ut", bufs=5)
                    nc.scalar.activation(ut, u_ps, AF.Gelu, bias=b1t[:, fc:fc + 1])
                    uts.append((k, ut))
                for ec in range(NC_D):
                    for k, ut in uts:
                        nc.tensor.matmul(z2_ps[ec], w2q[:, k, ec * P:(ec + 1) * P], ut,
                                         start=(g == 0 and k == 0),
                                         stop=(g == 3 and k == 3))
            for ec in range(NC_D):
                x3_c = data.tile([P, T], F32R, name="x3_c", tag="x", bufs=6)
                nc.vector.scalar_tensor_tensor(
                    out=x3_c, in0=z2_ps[ec], scalar=b2t[:, ec:ec + 1], in1=xs2[ec],
                    op0=ALU.add, op1=ALU.add)
                xs3.append(x3_c)
            xs = xs3

        # ---- final outputs ----
        for c in range(NC_D):
            nc.sync.dma_start(out=xT_out.ap()[c * P:(c + 1) * P, :], in_=xs[c])
        red2 = const.tile([1, 2], F32)
        nc.vector.tensor_reduce(out=red2[0:1, 0:1], in_=aux_sse,
                                axis=mybir.AxisListType.X, op=ALU.add)
        nc.vector.tensor_reduce(out=red2[0:1, 1:2], in_=ae_sse,
                                axis=mybir.AxisListType.X, op=ALU.add)
        nc.sync.dma_start(out=loss_out.ap(), in_=red2)

    nc.compile()
    return nc


# Fix for layernorm g-column passing: we wrap to supply per-chunk g columns.
# (layernorm takes gcol tile [P, NC_D]; slices internally per chunk)


_PROGRAM = None


def _get_program():
    global _PROGRAM
    if _PROGRAM is None:
        _PROGRAM = _build_program()
    return _PROGRAM


# ---------------------------------------------------------------------------
# Host wrapper
# ---------------------------------------------------------------------------

def kernel(seq=None, mask=None, mems=None, cmems=None, embed=None,
           ln1_g=None, ln1_b=None, wq=None, wkv=None, wo=None, bo=None,
           conv_w=None, conv_b=None, deconv_w=None, deconv_b=None,
           ln2_g=None, ln2_b=None, w1=None, b1=None, w2=None, b2=None):
    seq = np.asarray(seq)
    mems = np.asarray(mems, dtype=np.float32)
    cmems = np.asarray(cmems, dtype=np.float32)
    embed = np.asarray(embed, dtype=np.float32)

    pe = _positional_encoding()
    # per-core (batch element) x0^T
    x0 = embed[seq] + pe[None]          # [B, T, D]
    x0T = _round_tf32(np.ascontiguousarray(x0.transpose(0, 2, 1)))  # [B, D, T]

    memsT = _round_tf32(np.ascontiguousarray(np.transpose(mems, (1, 0, 3, 2))))   # [B,N,D,MEM]
    cmemsT = _round_tf32(np.ascontiguousarray(np.transpose(cmems, (1, 0, 3, 2))))  # [B,N,D,CMEM]

    wkv = np.asarray(wkv, dtype=np.float32)
    wk_h, wv_h = wkv[:, :, :D], wkv[:, :, D:]
    cw = np.asarray(conv_w, dtype=np.float32)      # [N, D(o), D(d), RATIO]
    cwm = cw.transpose(0, 3, 2, 1).reshape(N, RATIO * D, D)  # rows (r, d) -> o

    common = {
        "wq": _round_tf32(wq),
        "wk": _round_tf32(wk_h),
        "wv": _round_tf32(wv_h),
        "wo": _round_tf32(wo),
        "cwm": _round_tf32(cwm),
        "dwm": _round_tf32(deconv_w),               # [N, 128, 512]
        "w1m": _round_tf32(w1),
        "w2m": _round_tf32(w2),
        "cbm": _round_tf32(np.asarray(conv_b, np.float32).reshape(N, 1, D)),
        "dbm": _round_tf32(np.asarray(deconv_b, np.float32).reshape(N, 1, MEM)),
        "gb1": _round_tf32(np.stack([ln1_g, ln1_b], axis=1)),   # [N, 2, D]
        "gb2": _round_tf32(np.stack([ln2_g, ln2_b], axis=1)),
        "g1c": np.asarray(ln1_g, np.float32),
        "g2c": np.asarray(ln2_g, np.float32),
        "boc": np.asarray(bo, np.float32),
        "b1c": np.asarray(b1, np.float32),
        "b2c": np.asarray(b2, np.float32),
    }

    in_maps = []
    for b in range(NCORES):
        m = dict(common)
        m["xT0"] = x0T[b]
        m["memT"] = memsT[b]
        m["cmemT"] = cmemsT[b]
        in_maps.append(m)

    _install_ntff_hook()
    nc = _get_program()
    trace = os.environ.get("BASS_KERNEL_TRACE", "0") == "1"
    res = run_bass_kernel_spmd(nc, in_maps, list(range(NCORES)), trace=trace)
    kernel.last_result = res

    x_out = np.empty((B, T, D), np.float32)
    nm = np.empty((N, B, MEM, D), np.float32)
    ncm = np.empty((N, B, CMEM, D), np.float32)
    aux_sum = np.empty(B, np.float32)
    ae_sum = np.empty(B, np.float32)
    for b in range(NCORES):
        r = res.results[b]
        x_out[b] = r["xT_out"].T
        nm[:, b] = r["yT_out"].transpose(0, 2, 1)
        ncm[:, b] = r["comp_out"]
        aux_sum[b] = r["loss_out"][0, 0]
        ae_sum[b] = r["loss_out"][0, 1]

    denom = H * DH * T  # 262144 elements per layer per batch elem (aux)
    attn_loss = np.float32(aux_sum.mean() / denom / N)
    ae_loss = np.float32(ae_sum.mean() / (MEM * D) / N)
    return (x_out, nm, ncm, attn_loss, ae_loss)
